# revision 2
# baseline (speedup 1.0000x reference)
"""Multi-head attention (B=4, T=2048, E=1024, H=16, D=64) on 8 TRN2 cores.

Sharding: core c handles batch b = c//2 and heads hg = c%2 (8 heads each).
Host sums the two partial out-projections per batch.

v3 vs v2:
  - DMA consolidation: one dma_start per tensor/block (HWDGE is a single
    serialized resource with ~625ns fixed cost per dma_start; 98 small
    DMAs cost 61us of pipe time and gated the first window).
  - p_work yields rebalanced so every filler step carries ~2 matmuls.
  - v-projection PSUM evacuations moved to ACT (DVE queueing behind RoPE
    ops stalled the window-0 v chains).
  - Final window A(3) runs two heads in an alternating-slot pipeline so
    each head's AV hides the other's exp latency; out-projections of
    qb<3 fill the rest, and qb=3's out-projection is split into
    (c0,c1)/(c2,c3) halves so the first half runs mid-window.
"""
import sys
import numpy as np
from collections import deque
from contextlib import ExitStack

try:
    import concourse  # noqa: F401
except ImportError:
    sys.path.insert(0, "/opt/trn_rl_repo")

import ml_dtypes  # noqa: E402
import concourse.tile as tile  # noqa: E402
from concourse import bacc, mybir  # noqa: E402
from concourse.bass_utils import run_bass_kernel_spmd  # noqa: E402

F32 = mybir.dt.float32
BF16 = mybir.dt.bfloat16
AF = mybir.ActivationFunctionType

B, T, E, H, D = 4, 2048, 1024, 16, 64
N_CORES = 8
HPC = 8            # heads per core
EC = HPC * D       # 512 head-columns per core
TB = 512           # t/q block
KC = 128           # k chunk
NTB = T // TB      # 4
NTT = T // KC      # 16
CCH = E // 128     # 8 contraction chunks for x projections
OCH = EC // 128    # 4 chunks of the per-core head-column dim
ROPE_BASE = 10000.0

_NC = None


class _Filler:
    """Queue of deferred emission steps with PE-ns cost weights:
    generators advance one yield per step, callables run once. run_ns()
    spends a PE-time budget so filler coverage spreads evenly instead of
    exhausting early."""

    def __init__(self):
        self.items = deque()

    def add_gen(self, gen, step_ns):
        self.items.append(("g", gen, None, step_ns))

    def add_call(self, fn, a, cost_ns):
        self.items.append(("c", fn, a, cost_ns))

    balance = 0.0

    def step(self):
        """Returns the PE-ns cost of the emitted step, or 0 if empty."""
        while self.items:
            kind, obj, a, cost = self.items[0]
            if kind == "g":
                try:
                    next(obj)
                    return cost
                except StopIteration:
                    self.items.popleft()
                    continue
            self.items.popleft()
            obj(*a)
            return cost
        return 0

    def run_ns(self, budget):
        """Credit `budget` PE-ns and emit items while in credit. Overdraft
        carries so coarse items don't starve later call sites."""
        self.balance += budget
        while self.balance > 0:
            c = self.step()
            if c == 0:
                self.balance = 0
                return
            self.balance -= c

    def drain(self):
        while self.step():
            pass


def _build():
    nc = bacc.Bacc("TRN2", target_bir_lowering=False, debug=False,
                   num_devices=N_CORES)
    ap = {}
    def din(name, shape, dt=BF16):
        ap[name] = nc.dram_tensor(name, shape, dt, kind="ExternalInput").ap()
    din("xT", [E, T])              # x[b].T
    din("wqT", [E, EC])            # Wq[cols,:].T
    din("wkT", [E, EC])
    din("wvT", [E, EC])
    din("woT", [EC, E])            # Wo[:,cols].T
    din("cosb", [128, T], F32)     # cos dup'd over 2 heads, [2*64, T]
    din("sinb", [128, T], F32)
    y = nc.dram_tensor("y", [T, E], BF16, kind="ExternalOutput").ap()

    with tile.TileContext(nc) as tc, ExitStack() as ctx:
        persist = ctx.enter_context(tc.tile_pool(name="persist", bufs=1))
        qT = persist.tile([128, OCH, T], BF16, tag="qT")
        kT = persist.tile([128, OCH, T], BF16, tag="kT")
        vv = persist.tile([128, NTT, HPC, D + 1], BF16, tag="vv")
        oT = persist.tile([128, OCH, T], BF16, tag="oT")
        wq_sb = persist.tile([128, CCH, EC], BF16, tag="wq")
        wk_sb = persist.tile([128, CCH, EC], BF16, tag="wk")
        wv_sb = persist.tile([128, CCH, EC], BF16, tag="wv")
        wo_sb = persist.tile([128, OCH, E], BF16, tag="wo")
        ones_sb = persist.tile([128, HPC], BF16, tag="ones")
        nc.vector.memset(ones_sb[:], 1.0)

        xt_pool = ctx.enter_context(tc.tile_pool(name="xt", bufs=2))
        cs_pool = ctx.enter_context(tc.tile_pool(name="cs", bufs=2))
        tmp_pool = ctx.enter_context(tc.tile_pool(name="tmp", bufs=2))
        e_pool = ctx.enter_context(tc.tile_pool(name="e", bufs=4))
        r_pool = ctx.enter_context(tc.tile_pool(name="r", bufs=3))
        b_pool = ctx.enter_context(tc.tile_pool(name="b", bufs=3))
        ysb_pool = ctx.enter_context(tc.tile_pool(name="ysb", bufs=3))
        yh_pool = ctx.enter_context(tc.tile_pool(name="yh", bufs=8))
        # PSUM budget (8 banks): acc 2 + o/sw 2 + s 2x2 = 8
        acc_pool = ctx.enter_context(
            tc.tile_pool(name="acc", bufs=2, space="PSUM"))
        os_pool = ctx.enter_context(
            tc.tile_pool(name="os", bufs=2, space="PSUM"))
        s_pool = ctx.enter_context(
            tc.tile_pool(name="s", bufs=2, space="PSUM"))

        xTr = ap["xT"].rearrange("(c p) t -> p c t", p=128)

        # initial DMAs: wk/xt0 interleaved in quarters so the first k chain
        # starts ~3us and chases chunk arrivals; everything else
        # whole-tensor (one HWDGE slot each)
        wk_src = ap["wkT"].rearrange("(c p) e -> p c e", p=128)
        xt0 = xt_pool.tile([128, CCH, TB], BF16, tag="xt")
        qc = CCH // 4
        for q_ in range(4):
            cs_ = slice(q_ * qc, (q_ + 1) * qc)
            nc.sync.dma_start(out=wk_sb[:, cs_, :], in_=wk_src[:, cs_, :])
            nc.sync.dma_start(out=xt0[:, cs_, :], in_=xTr[:, cs_, 0:TB])
        cos0 = cs_pool.tile([128, TB], F32, tag="cos")
        sin0 = cs_pool.tile([128, TB], F32, tag="sin")
        nc.sync.dma_start(out=cos0, in_=ap["cosb"][:, 0:TB])
        nc.sync.dma_start(out=sin0, in_=ap["sinb"][:, 0:TB])
        nc.sync.dma_start(
            out=wq_sb, in_=ap["wqT"].rearrange("(c p) e -> p c e", p=128))
        nc.sync.dma_start(
            out=wv_sb, in_=ap["wvT"].rearrange("(c p) e -> p c e", p=128))
        nc.sync.dma_start(
            out=wo_sb, in_=ap["woT"].rearrange("(c p) e -> p c e", p=128))

        def dma_block(tb):
            ts = slice(tb * TB, (tb + 1) * TB)
            xt = xt_pool.tile([128, CCH, TB], BF16, tag="xt")
            nc.sync.dma_start(out=xt[:], in_=xTr[:, :, ts])
            cos_sb = cs_pool.tile([128, TB], F32, tag="cos")
            sin_sb = cs_pool.tile([128, TB], F32, tag="sin")
            nc.sync.dma_start(out=cos_sb, in_=ap["cosb"][:, ts])
            nc.sync.dma_start(out=sin_sb, in_=ap["sinb"][:, ts])
            return xt, cos_sb, sin_sb

        def p_work(tb, xt, cos_sb, sin_sb):
            """P(tb): k/q/v projections + RoPE for t-block tb; every yield
            boundary carries ~2 matmuls of PE work."""
            ts = slice(tb * TB, (tb + 1) * TB)
            pending_rope = []

            def emit_rope(dst, m):
                qs = tmp_pool.tile([128, TB], BF16, tag="qs")
                nc.vector.stream_shuffle(
                    qs[:], dst[:, m, ts],
                    mask=list(range(16, 32)) + list(range(0, 16)))
                t1 = tmp_pool.tile([128, TB], F32, tag="t1")
                nc.vector.tensor_mul(t1[:], dst[:, m, ts], cos_sb[:])
                t2 = tmp_pool.tile([128, TB], F32, tag="t2")
                nc.vector.tensor_mul(t2[:], qs[:], sin_sb[:])
                nc.vector.tensor_add(dst[:, m, ts], t1[:], t2[:])

            # K first so attention on later windows never waits on it
            for w_sb, dst in ((wk_sb, kT), (wq_sb, qT)):
                for m in range(OCH):
                    pp = acc_pool.tile([128, TB], F32, tag="acc")
                    for c in range(CCH):
                        nc.tensor.matmul(
                            pp[:], w_sb[:, c, m * 128:(m + 1) * 128],
                            xt[:, c, :], start=(c == 0), stop=(c == CCH - 1))
                        if c in (1, 3, 5):
                            yield
                    nc.scalar.activation(dst[:, m, ts], pp[:], AF.Copy)
                    # RoPE for chain m deferred behind chain m+1's matmuls
                    # so the swap matmul never stalls PE on the evacuation
                    pending_rope.append((dst, m))
                    if len(pending_rope) > 1:
                        emit_rope(*pending_rope.pop(0))
                    yield
            for st in range(TB // KC):
                tt = tb * (TB // KC) + st
                pp = acc_pool.tile([128, EC], F32, tag="acc")
                for c in range(CCH):
                    nc.tensor.matmul(
                        pp[:], xt[:, c, st * 128:(st + 1) * 128],
                        wv_sb[:, c, :], start=(c == 0), stop=(c == CCH - 1))
                    if c in (1, 3, 5):
                        yield
                nc.scalar.activation(
                    vv[:, tt, :, 0:D],
                    pp[:].rearrange("p (h d) -> p h d", d=D), AF.Copy)
                nc.vector.tensor_copy(vv[:, tt, :, D], ones_sb[:])
                yield
            for args in pending_rope:
                emit_rope(*args)

        # ---------------- attention pieces ----------------
        def qk_pair(qb, h, g):
            po = (h % 2) * 64
            ch = h // 2
            s_ps = s_pool.tile([128, 2, TB], F32, tag="s")
            for i in (0, 1):
                kc = 2 * g + i
                j = kc - 4 * qb
                co = 0 if j <= 0 else 128 * j
                nc.tensor.matmul(
                    s_ps[:, i, co:],
                    kT[po:po + D, ch, kc * KC:(kc + 1) * KC],
                    qT[po:po + D, ch, qb * TB + co:(qb + 1) * TB],
                    start=True, stop=True)
            return s_ps

        def exp_pair(qb, g, s_ps):
            e_sb = e_pool.tile([128, 2, TB], BF16, tag="e")
            if 2 * g + 1 < 4 * qb:
                # fully off-diagonal pair: one fused 1024-wide exp
                nc.scalar.activation(e_sb[:], s_ps[:], AF.Exp, scale=0.125)
            else:
                for i in (0, 1):
                    kc = 2 * g + i
                    j = kc - 4 * qb
                    co = 0 if j <= 0 else 128 * j
                    nc.scalar.activation(
                        e_sb[:, i, co:], s_ps[:, i, co:], AF.Exp, scale=0.125)
                    # zero the triangle inside the 128-wide block where the
                    # causal boundary passes; cols left of co are never read
                    nc.gpsimd.affine_select(
                        out=e_sb[:, i, co:co + KC],
                        in_=e_sb[:, i, co:co + KC],
                        compare_op=mybir.AluOpType.is_ge,
                        fill=0.0, base=0, pattern=[[1, KC]],
                        channel_multiplier=-1)
            return e_sb

        def av_pair(qb, h, o_ps, g, e_sb):
            nkv = (qb + 1) * 4
            for i in (0, 1):
                kc = 2 * g + i
                j = kc - 4 * qb
                co = 0 if j <= 0 else 128 * j
                nc.tensor.matmul(
                    o_ps[:, co:], vv[:, kc, h, :], e_sb[:, i, co:],
                    start=(kc == 0), stop=(kc == nkv - 1))

        def divide(qb, h, o_ps, split=1):
            po = (h % 2) * 64
            ch = h // 2
            r_sb = r_pool.tile([1, TB], F32, tag="r")
            nc.vector.reciprocal(r_sb[:], o_ps[D:D + 1, :])
            rb_sb = b_pool.tile([D, TB], F32, tag="rb")
            nc.gpsimd.partition_broadcast(rb_sb[:], r_sb[:])
            w = TB // split
            for p_ in range(split):
                cs_ = slice(p_ * w, (p_ + 1) * w)
                nc.vector.tensor_mul(
                    oT[po:po + D, ch, qb * TB + p_ * w:qb * TB + (p_ + 1) * w],
                    o_ps[0:D, cs_], rb_sb[:, cs_])

        def emit_attention(qb, fill, slot_ns, boundary_ns, init_ns=500):
            """Two heads in an alternating-slot pipeline: each head's QK/AV
            covers the other head's exp latency."""
            npair = (qb + 1) * 2
            for hh in (0, 2, 4, 6):
                heads = (hh, hh + 1)
                o_ps = {h: os_pool.tile([D + 1, TB], F32, tag="os",
                                        name=f"o{h}")
                        for h in heads}
                s_cur = {}
                s_cur[hh] = qk_pair(qb, hh, 0)
                fill.run_ns(init_ns)
                s_cur[hh + 1] = qk_pair(qb, hh + 1, 0)
                fill.run_ns(init_ns)
                for g in range(npair):
                    for h in heads:
                        e_cur = exp_pair(qb, g, s_cur[h])
                        if g + 1 < npair:
                            s_cur[h] = qk_pair(qb, h, g + 1)
                        fill.run_ns(slot_ns)
                        av_pair(qb, h, o_ps[h], g, e_cur)
                for h in heads:
                    divide(qb, h, o_ps[h])
                fill.run_ns(boundary_ns)

        def emit_yproj(qb, st, eh, evac_act=False):
            tt = qb * (TB // KC) + st
            tsl = slice(tt * 128, (tt + 1) * 128)
            y_ps = acc_pool.tile([128, 512], F32, tag="acc")
            for c in range(OCH):
                nc.tensor.matmul(
                    y_ps[:], oT[:, c, tsl],
                    wo_sb[:, c, eh * 512:(eh + 1) * 512],
                    start=(c == 0), stop=(c == OCH - 1))
            y_sb = ysb_pool.tile([128, 512], BF16, tag="ysb")
            if evac_act:
                # tail chains: ACT is idle there and, unlike DVE, its queue
                # is not behind the final divides — PSUM slots free sooner
                nc.scalar.activation(y_sb[:], y_ps[:], AF.Copy)
            else:
                nc.vector.tensor_copy(y_sb[:], y_ps[:])
            nc.sync.dma_start(
                out=y[tsl, eh * 512:(eh + 1) * 512], in_=y_sb[:])

        yhacc = {}

        def emit_ypart(st, eh, cs_, first):
            # partial out-projection of q-block 3 over oT chunks cs_,
            # accumulated into an SBUF f32 tile as head-pairs complete
            tt = 3 * (TB // KC) + st
            tsl = slice(tt * 128, (tt + 1) * 128)
            y_ps = acc_pool.tile([128, 512], F32, tag="acc")
            for k, c in enumerate(cs_):
                nc.tensor.matmul(
                    y_ps[:], oT[:, c, tsl],
                    wo_sb[:, c, eh * 512:(eh + 1) * 512],
                    start=(k == 0), stop=(k == len(cs_) - 1))
            if first:
                yh = yh_pool.tile([128, 512], F32, tag="yh")
                nc.vector.tensor_copy(yh[:], y_ps[:])
                yhacc[(st, eh)] = yh
            else:
                yh = yhacc[(st, eh)]
                nc.vector.tensor_add(yh[:], y_ps[:], yh[:])

        def emit_yfinal(st, eh):
            tt = 3 * (TB // KC) + st
            tsl = slice(tt * 128, (tt + 1) * 128)
            y_ps = acc_pool.tile([128, 512], F32, tag="acc")
            nc.tensor.matmul(
                y_ps[:], oT[:, 3, tsl], wo_sb[:, 3, eh * 512:(eh + 1) * 512],
                start=True, stop=True)
            y_sb = ysb_pool.tile([128, 512], BF16, tag="ysb")
            nc.vector.tensor_add(y_sb[:], y_ps[:], yhacc[(st, eh)][:])
            nc.sync.dma_start(
                out=y[tsl, eh * 512:(eh + 1) * 512], in_=y_sb[:])

        def emit_attention_final(fill):
            """A(3): two heads in an alternating-slot pipeline; each head's
            QK/AV covers the other head's exp latency. Partial
            out-projections of q-block 3 are emitted as chunks finalize."""
            qb = NTB - 1
            npair = (qb + 1) * 2
            for hh in (0, 2, 4, 6):
                heads = (hh, hh + 1)
                o_ps = {h: os_pool.tile([D + 1, TB], F32, tag="os",
                                        name=f"o{h}")
                        for h in heads}
                s_cur = {}
                s_cur[hh] = qk_pair(qb, hh, 0)
                fill.run_ns(500)
                s_cur[hh + 1] = qk_pair(qb, hh + 1, 0)
                fill.run_ns(500)
                for g in range(npair):
                    for h in heads:
                        e_cur = exp_pair(qb, g, s_cur[h])
                        if g + 1 < npair:
                            s_cur[h] = qk_pair(qb, h, g + 1)
                        fill.run_ns(200)
                        av_pair(qb, h, o_ps[h], g, e_cur)
                for h in heads:
                    divide(qb, h, o_ps[h])
                if hh != 6:
                    fill.run_ns(2000)

        # ---------------- window loop ----------------
        nxt = (xt0, cos0, sin0)
        per_w = {0: (350, 800), 1: (300, 700), 2: (260, 600)}
        for tb in range(NTB):
            xt, cos_sb, sin_sb = nxt
            if tb + 1 < NTB:
                nxt = dma_block(tb + 1)
            fill = _Filler()
            fill.add_gen(p_work(tb, xt, cos_sb, sin_sb), 500)
            if tb == 0:
                fill.drain()
            else:
                sl_, bd_ = per_w[tb - 1]
                emit_attention(tb - 1, fill, sl_, bd_)
                fill.drain()
        # final window: A(3) + all deferred out-projections as filler
        fill = _Filler()
        for qb in range(NTB - 1):
            for st in range(TB // KC):
                for eh in range(2):
                    fill.add_call(emit_yproj, (qb, st, eh), 853)
        emit_attention(NTB - 1, fill, 0, 2000)
        fill.drain()
        for st in range(TB // KC):
            for eh in range(2):
                emit_yproj(3, st, eh, evac_act=True)
    nc.compile()
    return nc


def _host_inputs(x, Wq, Wk, Wv, Wo):
    # rope tables in [e, t] layout, duplicated across the 2 heads of a chunk
    inv_freq = 1.0 / (ROPE_BASE ** (np.arange(0, D, 2, dtype=np.float64) / D))
    freqs = np.outer(np.arange(T, dtype=np.float64), inv_freq)  # [T, 32]
    emb = np.concatenate([freqs, freqs], axis=-1)               # [T, 64]
    cos1, sin1 = np.cos(emb).T, np.sin(emb).T                   # [64, T]
    cosb = np.concatenate([cos1, cos1], 0).astype(np.float32)   # [128, T]
    sinb = np.concatenate([sin1, sin1], 0).astype(np.float32)

    # head-dim permutation: rope pairs (i, i+32) land 16 apart within a
    # 32-partition quadrant so rotate-half is one DVE stream_shuffle.
    # Scores are invariant as long as q and k share the permutation.
    perm = np.concatenate([np.arange(0, 16), np.arange(32, 48),
                           np.arange(16, 32), np.arange(48, 64)])
    cos1, sin1 = cos1[perm], sin1[perm]
    sgn = np.where(np.arange(64) % 32 < 16, -1.0, 1.0)[:, None]
    sin1 = sin1 * sgn
    cosb = np.concatenate([cos1, cos1], 0).astype(np.float32)
    sinb = np.concatenate([sin1, sin1], 0).astype(np.float32)
    pidx = np.concatenate([64 * g + perm for g in range(HPC)])

    def bf(a):
        return np.ascontiguousarray(a).astype(ml_dtypes.bfloat16)

    xTs = [bf(x[b].T) for b in range(B)]
    wmaps = []
    for hg in range(2):
        cols = slice(hg * EC, (hg + 1) * EC)
        wmaps.append({
            "wqT": bf(Wq[cols, :][pidx].T),
            "wkT": bf(Wk[cols, :][pidx].T),
            "wvT": bf(Wv[cols, :].T),
            "woT": bf(Wo[:, cols].T),
        })
    in_maps = []
    for c in range(N_CORES):
        b, hg = c // 2, c % 2
        in_maps.append({
            "xT": xTs[b], "cosb": cosb, "sinb": sinb,
            **wmaps[hg],
        })
    return in_maps


def kernel(x, causal_mask, Wq, Wk, Wv, Wo):
    global _NC
    x = np.asarray(x, dtype=np.float32)
    Wq = np.asarray(Wq, dtype=np.float32)
    Wk = np.asarray(Wk, dtype=np.float32)
    Wv = np.asarray(Wv, dtype=np.float32)
    Wo = np.asarray(Wo, dtype=np.float32)
    if _NC is None:
        _NC = _build()
    in_maps = _host_inputs(x, Wq, Wk, Wv, Wo)
    try:
        res = run_bass_kernel_spmd(_NC, in_maps, list(range(N_CORES)))
    except Exception:
        # transient NRT/device hiccups recover on retry
        import time
        time.sleep(2)
        res = run_bass_kernel_spmd(_NC, in_maps, list(range(N_CORES)))
    out = np.empty((B, T, E), dtype=np.float32)
    for b in range(B):
        out[b] = (res.results[2 * b]["y"].astype(np.float32)
                  + res.results[2 * b + 1]["y"].astype(np.float32))
    return out


# revision 3
# speedup vs baseline: 1.0036x; 1.0036x over previous
"""Multi-head attention (B=4, T=2048, E=1024, H=16, D=64) on 8 TRN2 cores.

Sharding: core c handles batch b = c//2 and heads hg = c%2 (8 heads each).
Host sums the two partial out-projections per batch.

v3 vs v2:
  - DMA consolidation: one dma_start per tensor/block (HWDGE is a single
    serialized resource with ~625ns fixed cost per dma_start; 98 small
    DMAs cost 61us of pipe time and gated the first window).
  - p_work yields rebalanced so every filler step carries ~2 matmuls.
  - v-projection PSUM evacuations moved to ACT (DVE queueing behind RoPE
    ops stalled the window-0 v chains).
  - Final window A(3) runs two heads in an alternating-slot pipeline so
    each head's AV hides the other's exp latency; out-projections of
    qb<3 fill the rest, and qb=3's out-projection is split into
    (c0,c1)/(c2,c3) halves so the first half runs mid-window.
"""
import sys
import numpy as np
from collections import deque
from contextlib import ExitStack

try:
    import concourse  # noqa: F401
except ImportError:
    sys.path.insert(0, "/opt/trn_rl_repo")

import ml_dtypes  # noqa: E402
import concourse.tile as tile  # noqa: E402
from concourse import bacc, mybir  # noqa: E402
from concourse.bass_utils import run_bass_kernel_spmd  # noqa: E402

F32 = mybir.dt.float32
BF16 = mybir.dt.bfloat16
AF = mybir.ActivationFunctionType

B, T, E, H, D = 4, 2048, 1024, 16, 64
N_CORES = 8
HPC = 8            # heads per core
EC = HPC * D       # 512 head-columns per core
TB = 512           # t/q block
KC = 128           # k chunk
NTB = T // TB      # 4
NTT = T // KC      # 16
CCH = E // 128     # 8 contraction chunks for x projections
OCH = EC // 128    # 4 chunks of the per-core head-column dim
ROPE_BASE = 10000.0

_NC = None


class _Filler:
    """Queue of deferred emission steps with PE-ns cost weights:
    generators advance one yield per step, callables run once. run_ns()
    spends a PE-time budget so filler coverage spreads evenly instead of
    exhausting early."""

    def __init__(self):
        self.items = deque()

    def add_gen(self, gen, step_ns):
        self.items.append(("g", gen, None, step_ns))

    def add_call(self, fn, a, cost_ns):
        self.items.append(("c", fn, a, cost_ns))

    balance = 0.0

    def step(self):
        """Returns the PE-ns cost of the emitted step, or 0 if empty."""
        while self.items:
            kind, obj, a, cost = self.items[0]
            if kind == "g":
                try:
                    next(obj)
                    return cost
                except StopIteration:
                    self.items.popleft()
                    continue
            self.items.popleft()
            obj(*a)
            return cost
        return 0

    def run_ns(self, budget):
        """Credit `budget` PE-ns and emit items while in credit. Overdraft
        carries so coarse items don't starve later call sites."""
        self.balance += budget
        while self.balance > 0:
            c = self.step()
            if c == 0:
                self.balance = 0
                return
            self.balance -= c

    def drain(self):
        while self.step():
            pass


def _build():
    nc = bacc.Bacc("TRN2", target_bir_lowering=False, debug=False,
                   num_devices=N_CORES)
    ap = {}
    def din(name, shape, dt=BF16):
        ap[name] = nc.dram_tensor(name, shape, dt, kind="ExternalInput").ap()
    din("xT", [E, T])              # x[b].T
    din("wqT", [E, EC])            # Wq[cols,:].T
    din("wkT", [E, EC])
    din("wvT", [E, EC])
    din("woT", [EC, E])            # Wo[:,cols].T
    din("cosb", [128, T], F32)     # cos dup'd over 2 heads, [2*64, T]
    din("sinb", [128, T], F32)
    y = nc.dram_tensor("y", [T, E], BF16, kind="ExternalOutput").ap()

    with tile.TileContext(nc) as tc, ExitStack() as ctx:
        persist = ctx.enter_context(tc.tile_pool(name="persist", bufs=1))
        qT = persist.tile([128, OCH, T], BF16, tag="qT")
        kT = persist.tile([128, OCH, T], BF16, tag="kT")
        vv = persist.tile([128, NTT, HPC, D + 1], BF16, tag="vv")
        oT = persist.tile([128, OCH, T], BF16, tag="oT")
        wq_sb = persist.tile([128, CCH, EC], BF16, tag="wq")
        wk_sb = persist.tile([128, CCH, EC], BF16, tag="wk")
        wv_sb = persist.tile([128, CCH, EC], BF16, tag="wv")
        wo_sb = persist.tile([128, OCH, E], BF16, tag="wo")
        ones_sb = persist.tile([128, HPC], BF16, tag="ones")
        nc.vector.memset(ones_sb[:], 1.0)

        xt_pool = ctx.enter_context(tc.tile_pool(name="xt", bufs=2))
        cs_pool = ctx.enter_context(tc.tile_pool(name="cs", bufs=2))
        tmp_pool = ctx.enter_context(tc.tile_pool(name="tmp", bufs=2))
        e_pool = ctx.enter_context(tc.tile_pool(name="e", bufs=6))
        r_pool = ctx.enter_context(tc.tile_pool(name="r", bufs=4))
        b_pool = ctx.enter_context(tc.tile_pool(name="b", bufs=4))
        ysb_pool = ctx.enter_context(tc.tile_pool(name="ysb", bufs=4))
        yh_pool = ctx.enter_context(tc.tile_pool(name="yh", bufs=8))
        # PSUM budget (8 banks): acc 2 + o/sw 2 + s 2x2 = 8
        acc_pool = ctx.enter_context(
            tc.tile_pool(name="acc", bufs=2, space="PSUM"))
        os_pool = ctx.enter_context(
            tc.tile_pool(name="os", bufs=2, space="PSUM"))
        s_pool = ctx.enter_context(
            tc.tile_pool(name="s", bufs=2, space="PSUM"))

        xTr = ap["xT"].rearrange("(c p) t -> p c t", p=128)

        # initial DMAs: wk/xt0 interleaved in quarters so the first k chain
        # starts ~3us and chases chunk arrivals; everything else
        # whole-tensor (one HWDGE slot each)
        wk_src = ap["wkT"].rearrange("(c p) e -> p c e", p=128)
        xt0 = xt_pool.tile([128, CCH, TB], BF16, tag="xt")
        qc = CCH // 4
        for q_ in range(4):
            cs_ = slice(q_ * qc, (q_ + 1) * qc)
            nc.sync.dma_start(out=wk_sb[:, cs_, :], in_=wk_src[:, cs_, :])
            nc.sync.dma_start(out=xt0[:, cs_, :], in_=xTr[:, cs_, 0:TB])
        cos0 = cs_pool.tile([128, TB], F32, tag="cos")
        sin0 = cs_pool.tile([128, TB], F32, tag="sin")
        nc.sync.dma_start(out=cos0, in_=ap["cosb"][:, 0:TB])
        nc.sync.dma_start(out=sin0, in_=ap["sinb"][:, 0:TB])
        nc.sync.dma_start(
            out=wq_sb, in_=ap["wqT"].rearrange("(c p) e -> p c e", p=128))
        nc.sync.dma_start(
            out=wv_sb, in_=ap["wvT"].rearrange("(c p) e -> p c e", p=128))
        nc.sync.dma_start(
            out=wo_sb, in_=ap["woT"].rearrange("(c p) e -> p c e", p=128))

        def dma_block(tb):
            ts = slice(tb * TB, (tb + 1) * TB)
            xt = xt_pool.tile([128, CCH, TB], BF16, tag="xt")
            nc.sync.dma_start(out=xt[:], in_=xTr[:, :, ts])
            cos_sb = cs_pool.tile([128, TB], F32, tag="cos")
            sin_sb = cs_pool.tile([128, TB], F32, tag="sin")
            nc.sync.dma_start(out=cos_sb, in_=ap["cosb"][:, ts])
            nc.sync.dma_start(out=sin_sb, in_=ap["sinb"][:, ts])
            return xt, cos_sb, sin_sb

        def p_work(tb, xt, cos_sb, sin_sb):
            """P(tb): k/q/v projections + RoPE for t-block tb; every yield
            boundary carries ~2 matmuls of PE work."""
            ts = slice(tb * TB, (tb + 1) * TB)
            pending_rope = []

            def emit_rope(dst, m):
                qs = tmp_pool.tile([128, TB], BF16, tag="qs")
                nc.vector.stream_shuffle(
                    qs[:], dst[:, m, ts],
                    mask=list(range(16, 32)) + list(range(0, 16)))
                t1 = tmp_pool.tile([128, TB], F32, tag="t1")
                nc.vector.tensor_mul(t1[:], dst[:, m, ts], cos_sb[:])
                t2 = tmp_pool.tile([128, TB], F32, tag="t2")
                nc.vector.tensor_mul(t2[:], qs[:], sin_sb[:])
                nc.vector.tensor_add(dst[:, m, ts], t1[:], t2[:])

            # K first so attention on later windows never waits on it
            for w_sb, dst in ((wk_sb, kT), (wq_sb, qT)):
                for m in range(OCH):
                    pp = acc_pool.tile([128, TB], F32, tag="acc")
                    for c in range(CCH):
                        nc.tensor.matmul(
                            pp[:], w_sb[:, c, m * 128:(m + 1) * 128],
                            xt[:, c, :], start=(c == 0), stop=(c == CCH - 1))
                        if c in (1, 3, 5):
                            yield
                    nc.scalar.activation(dst[:, m, ts], pp[:], AF.Copy)
                    # RoPE for chain m deferred behind chain m+1's matmuls
                    # so the swap matmul never stalls PE on the evacuation
                    pending_rope.append((dst, m))
                    if len(pending_rope) > 1:
                        emit_rope(*pending_rope.pop(0))
                    yield
            for st in range(TB // KC):
                tt = tb * (TB // KC) + st
                pp = acc_pool.tile([128, EC], F32, tag="acc")
                for c in range(CCH):
                    nc.tensor.matmul(
                        pp[:], xt[:, c, st * 128:(st + 1) * 128],
                        wv_sb[:, c, :], start=(c == 0), stop=(c == CCH - 1))
                    if c in (1, 3, 5):
                        yield
                nc.scalar.activation(
                    vv[:, tt, :, 0:D],
                    pp[:].rearrange("p (h d) -> p h d", d=D), AF.Copy)
                nc.vector.tensor_copy(vv[:, tt, :, D], ones_sb[:])
                yield
            for args in pending_rope:
                emit_rope(*args)

        # ---------------- attention pieces ----------------
        def qk_pair(qb, h, g):
            po = (h % 2) * 64
            ch = h // 2
            s_ps = s_pool.tile([128, 2, TB], F32, tag="s")
            for i in (0, 1):
                kc = 2 * g + i
                j = kc - 4 * qb
                co = 0 if j <= 0 else 128 * j
                nc.tensor.matmul(
                    s_ps[:, i, co:],
                    kT[po:po + D, ch, kc * KC:(kc + 1) * KC],
                    qT[po:po + D, ch, qb * TB + co:(qb + 1) * TB],
                    start=True, stop=True)
            return s_ps

        def exp_pair(qb, g, s_ps):
            e_sb = e_pool.tile([128, 2, TB], BF16, tag="e")
            if 2 * g + 1 < 4 * qb:
                # fully off-diagonal pair: one fused 1024-wide exp
                nc.scalar.activation(e_sb[:], s_ps[:], AF.Exp, scale=0.125)
            else:
                for i in (0, 1):
                    kc = 2 * g + i
                    j = kc - 4 * qb
                    co = 0 if j <= 0 else 128 * j
                    nc.scalar.activation(
                        e_sb[:, i, co:], s_ps[:, i, co:], AF.Exp, scale=0.125)
                    # zero the triangle inside the 128-wide block where the
                    # causal boundary passes; cols left of co are never read
                    nc.gpsimd.affine_select(
                        out=e_sb[:, i, co:co + KC],
                        in_=e_sb[:, i, co:co + KC],
                        compare_op=mybir.AluOpType.is_ge,
                        fill=0.0, base=0, pattern=[[1, KC]],
                        channel_multiplier=-1)
            return e_sb

        def av_pair(qb, h, o_ps, g, e_sb):
            nkv = (qb + 1) * 4
            for i in (0, 1):
                kc = 2 * g + i
                j = kc - 4 * qb
                co = 0 if j <= 0 else 128 * j
                nc.tensor.matmul(
                    o_ps[:, co:], vv[:, kc, h, :], e_sb[:, i, co:],
                    start=(kc == 0), stop=(kc == nkv - 1))

        def divide(qb, h, o_ps, split=1):
            po = (h % 2) * 64
            ch = h // 2
            r_sb = r_pool.tile([1, TB], F32, tag="r")
            nc.vector.reciprocal(r_sb[:], o_ps[D:D + 1, :])
            rb_sb = b_pool.tile([D, TB], F32, tag="rb")
            nc.gpsimd.partition_broadcast(rb_sb[:], r_sb[:])
            w = TB // split
            for p_ in range(split):
                cs_ = slice(p_ * w, (p_ + 1) * w)
                nc.vector.tensor_mul(
                    oT[po:po + D, ch, qb * TB + p_ * w:qb * TB + (p_ + 1) * w],
                    o_ps[0:D, cs_], rb_sb[:, cs_])

        def emit_attention(qb, fill, slot_ns, boundary_ns, init_ns=500):
            """Two heads in an alternating-slot pipeline: each head's QK/AV
            covers the other head's exp latency."""
            npair = (qb + 1) * 2
            for hh in (0, 2, 4, 6):
                heads = (hh, hh + 1)
                o_ps = {h: os_pool.tile([D + 1, TB], F32, tag="os",
                                        name=f"o{h}")
                        for h in heads}
                s_cur = {}
                s_cur[hh] = qk_pair(qb, hh, 0)
                fill.run_ns(init_ns)
                s_cur[hh + 1] = qk_pair(qb, hh + 1, 0)
                fill.run_ns(init_ns)
                for g in range(npair):
                    for h in heads:
                        e_cur = exp_pair(qb, g, s_cur[h])
                        if g + 1 < npair:
                            s_cur[h] = qk_pair(qb, h, g + 1)
                        fill.run_ns(slot_ns)
                        av_pair(qb, h, o_ps[h], g, e_cur)
                for h in heads:
                    divide(qb, h, o_ps[h])
                fill.run_ns(boundary_ns)

        def emit_yproj(qb, st, eh, evac_act=False):
            tt = qb * (TB // KC) + st
            tsl = slice(tt * 128, (tt + 1) * 128)
            y_ps = acc_pool.tile([128, 512], F32, tag="acc")
            for c in range(OCH):
                nc.tensor.matmul(
                    y_ps[:], oT[:, c, tsl],
                    wo_sb[:, c, eh * 512:(eh + 1) * 512],
                    start=(c == 0), stop=(c == OCH - 1))
            y_sb = ysb_pool.tile([128, 512], BF16, tag="ysb")
            if evac_act:
                # tail chains: ACT is idle there and, unlike DVE, its queue
                # is not behind the final divides — PSUM slots free sooner
                nc.scalar.activation(y_sb[:], y_ps[:], AF.Copy)
            else:
                nc.vector.tensor_copy(y_sb[:], y_ps[:])
            nc.sync.dma_start(
                out=y[tsl, eh * 512:(eh + 1) * 512], in_=y_sb[:])

        yhacc = {}

        def emit_ypart(st, eh, cs_, first):
            # partial out-projection of q-block 3 over oT chunks cs_,
            # accumulated into an SBUF f32 tile as head-pairs complete
            tt = 3 * (TB // KC) + st
            tsl = slice(tt * 128, (tt + 1) * 128)
            y_ps = acc_pool.tile([128, 512], F32, tag="acc")
            for k, c in enumerate(cs_):
                nc.tensor.matmul(
                    y_ps[:], oT[:, c, tsl],
                    wo_sb[:, c, eh * 512:(eh + 1) * 512],
                    start=(k == 0), stop=(k == len(cs_) - 1))
            if first:
                yh = yh_pool.tile([128, 512], F32, tag="yh")
                nc.vector.tensor_copy(yh[:], y_ps[:])
                yhacc[(st, eh)] = yh
            else:
                yh = yhacc[(st, eh)]
                nc.vector.tensor_add(yh[:], y_ps[:], yh[:])

        def emit_yfinal(st, eh):
            tt = 3 * (TB // KC) + st
            tsl = slice(tt * 128, (tt + 1) * 128)
            y_ps = acc_pool.tile([128, 512], F32, tag="acc")
            nc.tensor.matmul(
                y_ps[:], oT[:, 3, tsl], wo_sb[:, 3, eh * 512:(eh + 1) * 512],
                start=True, stop=True)
            y_sb = ysb_pool.tile([128, 512], BF16, tag="ysb")
            nc.vector.tensor_add(y_sb[:], y_ps[:], yhacc[(st, eh)][:])
            nc.sync.dma_start(
                out=y[tsl, eh * 512:(eh + 1) * 512], in_=y_sb[:])

        def emit_attention_final(fill):
            """A(3): two heads in an alternating-slot pipeline; each head's
            QK/AV covers the other head's exp latency. Partial
            out-projections of q-block 3 are emitted as chunks finalize."""
            qb = NTB - 1
            npair = (qb + 1) * 2
            for hh in (0, 2, 4, 6):
                heads = (hh, hh + 1)
                o_ps = {h: os_pool.tile([D + 1, TB], F32, tag="os",
                                        name=f"o{h}")
                        for h in heads}
                s_cur = {}
                s_cur[hh] = qk_pair(qb, hh, 0)
                fill.run_ns(500)
                s_cur[hh + 1] = qk_pair(qb, hh + 1, 0)
                fill.run_ns(500)
                for g in range(npair):
                    for h in heads:
                        e_cur = exp_pair(qb, g, s_cur[h])
                        if g + 1 < npair:
                            s_cur[h] = qk_pair(qb, h, g + 1)
                        fill.run_ns(200)
                        av_pair(qb, h, o_ps[h], g, e_cur)
                for h in heads:
                    divide(qb, h, o_ps[h])
                if hh != 6:
                    fill.run_ns(2000)

        # ---------------- window loop ----------------
        nxt = (xt0, cos0, sin0)
        per_w = {0: (350, 800), 1: (300, 700), 2: (260, 600)}
        for tb in range(NTB):
            xt, cos_sb, sin_sb = nxt
            if tb + 1 < NTB:
                nxt = dma_block(tb + 1)
            fill = _Filler()
            fill.add_gen(p_work(tb, xt, cos_sb, sin_sb), 500)
            if tb == 0:
                fill.drain()
            else:
                sl_, bd_ = per_w[tb - 1]
                emit_attention(tb - 1, fill, sl_, bd_)
                fill.drain()
        # final window: A(3) + all deferred out-projections as filler
        fill = _Filler()
        n_ = 0
        for qb in range(NTB - 1):
            for st in range(TB // KC):
                for eh in range(2):
                    # the last fillers run next to the final divides: ACT
                    # evacuation keeps their PSUM release off the DVE queue
                    fill.add_call(emit_yproj, (qb, st, eh, n_ >= 20), 853)
                    n_ += 1
        emit_attention(NTB - 1, fill, 0, 2000)
        fill.drain()
        for st in range(TB // KC):
            for eh in range(2):
                emit_yproj(3, st, eh, evac_act=True)
    nc.compile()
    return nc


def _host_inputs(x, Wq, Wk, Wv, Wo):
    # rope tables in [e, t] layout, duplicated across the 2 heads of a chunk
    inv_freq = 1.0 / (ROPE_BASE ** (np.arange(0, D, 2, dtype=np.float64) / D))
    freqs = np.outer(np.arange(T, dtype=np.float64), inv_freq)  # [T, 32]
    emb = np.concatenate([freqs, freqs], axis=-1)               # [T, 64]
    cos1, sin1 = np.cos(emb).T, np.sin(emb).T                   # [64, T]
    cosb = np.concatenate([cos1, cos1], 0).astype(np.float32)   # [128, T]
    sinb = np.concatenate([sin1, sin1], 0).astype(np.float32)

    # head-dim permutation: rope pairs (i, i+32) land 16 apart within a
    # 32-partition quadrant so rotate-half is one DVE stream_shuffle.
    # Scores are invariant as long as q and k share the permutation.
    perm = np.concatenate([np.arange(0, 16), np.arange(32, 48),
                           np.arange(16, 32), np.arange(48, 64)])
    cos1, sin1 = cos1[perm], sin1[perm]
    sgn = np.where(np.arange(64) % 32 < 16, -1.0, 1.0)[:, None]
    sin1 = sin1 * sgn
    cosb = np.concatenate([cos1, cos1], 0).astype(np.float32)
    sinb = np.concatenate([sin1, sin1], 0).astype(np.float32)
    pidx = np.concatenate([64 * g + perm for g in range(HPC)])

    def bf(a):
        return np.ascontiguousarray(a).astype(ml_dtypes.bfloat16)

    xTs = [bf(x[b].T) for b in range(B)]
    wmaps = []
    for hg in range(2):
        cols = slice(hg * EC, (hg + 1) * EC)
        wmaps.append({
            "wqT": bf(Wq[cols, :][pidx].T),
            "wkT": bf(Wk[cols, :][pidx].T),
            "wvT": bf(Wv[cols, :].T),
            "woT": bf(Wo[:, cols].T),
        })
    in_maps = []
    for c in range(N_CORES):
        b, hg = c // 2, c % 2
        in_maps.append({
            "xT": xTs[b], "cosb": cosb, "sinb": sinb,
            **wmaps[hg],
        })
    return in_maps


def kernel(x, causal_mask, Wq, Wk, Wv, Wo):
    global _NC
    x = np.asarray(x, dtype=np.float32)
    Wq = np.asarray(Wq, dtype=np.float32)
    Wk = np.asarray(Wk, dtype=np.float32)
    Wv = np.asarray(Wv, dtype=np.float32)
    Wo = np.asarray(Wo, dtype=np.float32)
    if _NC is None:
        _NC = _build()
    in_maps = _host_inputs(x, Wq, Wk, Wv, Wo)
    try:
        res = run_bass_kernel_spmd(_NC, in_maps, list(range(N_CORES)))
    except Exception:
        # transient NRT/device hiccups recover on retry
        import time
        time.sleep(2)
        res = run_bass_kernel_spmd(_NC, in_maps, list(range(N_CORES)))
    out = np.empty((B, T, E), dtype=np.float32)
    for b in range(B):
        out[b] = (res.results[2 * b]["y"].astype(np.float32)
                  + res.results[2 * b + 1]["y"].astype(np.float32))
    return out
